# revision 5
# baseline (speedup 1.0000x reference)
"""Bass/Trainium2 kernel for nn_GCL_49959059587771 (GCL JSD loss).

Math: reference computes, for pair (z, g, batch):
    zn = z / max(||z||, eps);  gn = g / max(||g||, eps)
    self_sim  = (zn @ gn_self.T)  * onehot(batch)   # [N, G]
    cross_sim = (zn @ gn_cross.T) * onehot(batch)
    d = ep_jsd(self_sim).sum(1) - ep_jsd(cross_sim).sum(1)
    L = sqrt(sum(d^2))
where ep_jsd(x) = log2 - softplus(-x), and ep_jsd(0) = 0 exactly. The
one-hot mask therefore collapses each row of the [N, G] matrices to a
single entry: the masked row-sum of ep_jsd equals ep_jsd of the one
gathered dot product (all other entries are ep_jsd(0) = 0, and even their
shared constant would cancel in the self-cross difference).

So per node i:
    s_i = <z_i, gn_self[b_i]>  / ||z_i||
    c_i = <z_i, gn_cross[b_i]> / ||z_i||
    d_i = softplus(-c_i) - softplus(-s_i)
and the answer is sqrt(sum d1^2) + sqrt(sum d2^2).

Strategy (8 NeuronCores, SPMD, node-parallel):
  - shard nodes N across 8 cores (6250 each, padded to 6272 = 49*128)
  - replicate g (concatenated [g1 | g2] rows -> one 512-col "cat row")
  - on device: normalize g rows once, store to DRAM scratch, then
    dma_gather cat rows per node; per 128-node tile use the fused DVE
    tensor_tensor_reduce (multiply + free-axis reduce) for the two dot
    products and ACT Square+accum for ||z||^2; tiny [128, 49] epilogue
    does the normalize / softplus / d^2 accumulation via Exp/Ln.
  - per-core output: [128, 2] partial sums of d1^2 / d2^2; host finishes
    the all-reduce (sum over cores+partitions), sqrt, add.
"""

import numpy as np
from contextlib import ExitStack

import concourse.bass as bass
import concourse.bacc as bacc
import concourse.tile as tile
import concourse.mybir as mybir
from concourse.bass_utils import run_bass_kernel_spmd

N, G, D = 50000, 512, 256
NCORES = 8
RPC = N // NCORES            # 6250 rows per core
NT = 49                      # 128-row tiles per core
RPAD = NT * 128              # 6272
GRP = 7                      # tiles per gather/DMA group
NGRP = NT // GRP             # 7

AF = mybir.ActivationFunctionType
ALU = mybir.AluOpType
F32 = mybir.dt.float32
I16 = mybir.dt.int16
BF16 = mybir.dt.bfloat16

# compute dtype for z tiles and normalized-g gather payloads
Z_DT = BF16
G_DT = BF16
_NP_Z = {F32: np.float32, BF16: "bfloat16"}


def build(z_dt=Z_DT, g_dt=G_DT, debug=False):
    nc = bacc.Bacc("TRN2", target_bir_lowering=False, debug=debug)

    z1t = nc.dram_tensor("z1t", [128, NT, D], z_dt, kind="ExternalInput")
    z2t = nc.dram_tensor("z2t", [128, NT, D], z_dt, kind="ExternalInput")
    i1 = nc.dram_tensor("i1", [128, RPAD // 16], I16, kind="ExternalInput")
    i2 = nc.dram_tensor("i2", [128, RPAD // 16], I16, kind="ExternalInput")
    gcat = nc.dram_tensor("gcat", [G, 2 * D], F32, kind="ExternalInput")
    gcat_n = nc.dram_tensor("gcat_n", [G, 2 * D], g_dt)
    acc = nc.dram_tensor("acc", [128, 2], F32, kind="ExternalOutput")

    with tile.TileContext(nc) as tc, ExitStack() as ctx:
        gpool = ctx.enter_context(tc.tile_pool(name="gnorm", bufs=2))
        singles = ctx.enter_context(tc.tile_pool(name="singles", bufs=1))
        zpool = ctx.enter_context(tc.tile_pool(name="z", bufs=3))
        ggpool = ctx.enter_context(tc.tile_pool(name="gg", bufs=3))
        junk = ctx.enter_context(tc.tile_pool(name="junk", bufs=6))
        small = ctx.enter_context(tc.tile_pool(name="small", bufs=4))

        # ---- phase 1: row-normalize g1|g2 cat rows, store to DRAM ----
        for gt in range(G // 128):
            g_in = gpool.tile([128, 2 * D], F32, tag="g_in")
            nc.sync.dma_start(g_in[:], gcat[gt * 128:(gt + 1) * 128, :])
            n2 = small.tile([128, 2], F32, tag="gn2")
            for h in range(2):
                sq = junk.tile([128, D], F32, tag="junk")
                nc.vector.scalar_tensor_tensor(
                    out=sq[:], in0=g_in[:, h * D:(h + 1) * D], scalar=1.0,
                    in1=g_in[:, h * D:(h + 1) * D],
                    op0=ALU.mult, op1=ALU.mult, accum_out=n2[:, h:h + 1])
            # 1/sqrt(n2) = exp(-0.5 * ln(n2)); g norms are O(16), no eps issue
            inv = small.tile([128, 2], F32, tag="ginv")
            nc.scalar.activation(out=inv[:], in_=n2[:], func=AF.Ln)
            nc.scalar.activation(out=inv[:], in_=inv[:], func=AF.Exp, scale=-0.5)
            g_out = gpool.tile([128, 2 * D], g_dt, tag="g_out")
            for h in range(2):
                nc.vector.tensor_scalar_mul(
                    g_out[:, h * D:(h + 1) * D], g_in[:, h * D:(h + 1) * D],
                    inv[:, h:h + 1])
            nc.sync.dma_start(gcat_n[gt * 128:(gt + 1) * 128, :], g_out[:])

        # ---- gather indices ----
        i1_sb = singles.tile([128, RPAD // 16], I16)
        nc.sync.dma_start(i1_sb[:], i1[:])
        i2_sb = singles.tile([128, RPAD // 16], I16)
        nc.sync.dma_start(i2_sb[:], i2[:])

        # ---- per-tile accumulators ----
        r1s = singles.tile([128, NT], F32)
        r1c = singles.tile([128, NT], F32)
        r2s = singles.tile([128, NT], F32)
        r2c = singles.tile([128, NT], F32)
        nn1 = singles.tile([128, NT], F32)
        nn2 = singles.tile([128, NT], F32)

        # ---- main loop ----
        iw = (GRP * 128) // 16  # idx words per group
        for grp in range(NGRP):
            gg1 = ggpool.tile([128, GRP, 2 * D], g_dt, tag="gg1")
            nc.gpsimd.dma_gather(gg1[:], gcat_n[:],
                                 i1_sb[:, grp * iw:(grp + 1) * iw],
                                 GRP * 128, GRP * 128, 2 * D)
            gg2 = ggpool.tile([128, GRP, 2 * D], g_dt, tag="gg2")
            nc.gpsimd.dma_gather(gg2[:], gcat_n[:],
                                 i2_sb[:, grp * iw:(grp + 1) * iw],
                                 GRP * 128, GRP * 128, 2 * D)
            z1c = zpool.tile([128, GRP, D], z_dt, tag="z1c")
            nc.sync.dma_start(z1c[:], z1t[:, grp * GRP:(grp + 1) * GRP, :])
            z2c = zpool.tile([128, GRP, D], z_dt, tag="z2c")
            nc.sync.dma_start(z2c[:], z2t[:, grp * GRP:(grp + 1) * GRP, :])
            for tt in range(GRP):
                t = grp * GRP + tt
                for (zc, gg, rs, rc, nn) in ((z1c, gg1, r1s, r1c, nn1),
                                             (z2c, gg2, r2s, r2c, nn2)):
                    # self dot: pair 1 uses g1n (cols 0:D), pair 2 uses g2n
                    sh, ch = (0, D) if zc is z1c else (D, 0)
                    js = junk.tile([128, D], z_dt, tag="junk")
                    nc.vector.scalar_tensor_tensor(
                        out=js[:], in0=zc[:, tt, :], scalar=1.0,
                        in1=gg[:, tt, sh:sh + D],
                        op0=ALU.mult, op1=ALU.mult, accum_out=rs[:, t:t + 1])
                    jc = junk.tile([128, D], z_dt, tag="junk")
                    nc.vector.scalar_tensor_tensor(
                        out=jc[:], in0=zc[:, tt, :], scalar=1.0,
                        in1=gg[:, tt, ch:ch + D],
                        op0=ALU.mult, op1=ALU.mult, accum_out=rc[:, t:t + 1])
                    jn = junk.tile([128, D], F32, tag="junk")
                    nc.scalar.activation(out=jn[:], in_=zc[:, tt, :],
                                         func=AF.Square,
                                         accum_out=nn[:, t:t + 1])

        # ---- epilogue on [128, NT] ----
        # inv_norm = exp(-0.5*ln(n2 + eps));  eps keeps padded zero rows finite
        eps_b = singles.tile([128, 1], F32)
        nc.vector.memset(eps_b[:], 1e-12)
        inv1 = singles.tile([128, NT], F32)
        nc.scalar.activation(out=inv1[:], in_=nn1[:], func=AF.Ln, bias=eps_b[:])
        nc.scalar.activation(out=inv1[:], in_=inv1[:], func=AF.Exp, scale=-0.5)
        inv2 = singles.tile([128, NT], F32)
        nc.scalar.activation(out=inv2[:], in_=nn2[:], func=AF.Ln, bias=eps_b[:])
        nc.scalar.activation(out=inv2[:], in_=inv2[:], func=AF.Exp, scale=-0.5)

        acc_sb = singles.tile([128, 2], F32)
        for j, (rs, rc, inv) in enumerate(((r1s, r1c, inv1), (r2s, r2c, inv2))):
            s = small.tile([128, NT], F32, tag="s")
            nc.vector.tensor_mul(s[:], rs[:], inv[:])
            c = small.tile([128, NT], F32, tag="c")
            nc.vector.tensor_mul(c[:], rc[:], inv[:])
            # softplus(-x) = ln(1 + exp(-x))
            sp_s = small.tile([128, NT], F32, tag="sp_s")
            nc.scalar.activation(out=sp_s[:], in_=s[:], func=AF.Exp, scale=-1.0)
            nc.scalar.activation(out=sp_s[:], in_=sp_s[:], func=AF.Ln, bias=1.0)
            sp_c = small.tile([128, NT], F32, tag="sp_c")
            nc.scalar.activation(out=sp_c[:], in_=c[:], func=AF.Exp, scale=-1.0)
            nc.scalar.activation(out=sp_c[:], in_=sp_c[:], func=AF.Ln, bias=1.0)
            d = small.tile([128, NT], F32, tag="d")
            nc.vector.tensor_sub(d[:], sp_c[:], sp_s[:])
            jd = junk.tile([128, NT], F32, tag="jd")
            nc.scalar.activation(out=jd[:], in_=d[:], func=AF.Square,
                                 accum_out=acc_sb[:, j:j + 1])
        nc.sync.dma_start(acc[:], acc_sb[:])

    nc.compile()
    return nc


_prog = None


def _get_prog():
    global _prog
    if _prog is None:
        _prog = build()
    return _prog


def _prep_inputs(z1, z2, g1, g2, batch_1, batch_2):
    z1 = np.asarray(z1, dtype=np.float32)
    z2 = np.asarray(z2, dtype=np.float32)
    b1 = np.asarray(batch_1).astype(np.int64).ravel()
    b2 = np.asarray(batch_2).astype(np.int64).ravel()
    gcat = np.ascontiguousarray(
        np.concatenate([np.asarray(g1, np.float32),
                        np.asarray(g2, np.float32)], axis=1))
    z_np = np.dtype(_NP_Z[Z_DT]) if Z_DT == F32 else np.dtype("bfloat16")
    import ml_dtypes  # noqa: F401  (registers bfloat16 with numpy)

    in_maps = []
    for k in range(NCORES):
        sl = slice(k * RPC, (k + 1) * RPC)

        def prep_z(z):
            zs = np.zeros((RPAD, D), np.float32)
            zs[:RPC] = z[sl]
            zt = zs.reshape(NT, 128, D).transpose(1, 0, 2)
            return np.ascontiguousarray(zt.astype(z_np))

        def prep_i(b):
            ii = np.zeros(RPAD, np.int64)
            ii[:RPC] = b[sl]
            w = ii.reshape(RPAD // 16, 16).T.astype(np.int16)  # [16, RPAD/16]
            return np.ascontiguousarray(np.tile(w, (8, 1)))    # [128, ...]

        in_maps.append({"z1t": prep_z(z1), "z2t": prep_z(z2),
                        "i1": prep_i(b1), "i2": prep_i(b2), "gcat": gcat})
    return in_maps


def _finish(results):
    tot = np.zeros(2, np.float64)
    for r in results:
        tot += r["acc"].astype(np.float64).sum(axis=0)
    return np.float32(np.sqrt(tot[0]) + np.sqrt(tot[1]))


def kernel(z1, z2, g1, g2, batch_1, batch_2, trace=False):
    nc = _get_prog()
    in_maps = _prep_inputs(z1, z2, g1, g2, batch_1, batch_2)
    res = run_bass_kernel_spmd(nc, in_maps, core_ids=list(range(NCORES)),
                               trace=trace)
    out = _finish(res.results)
    if trace:
        kernel.last_results = res
    return out


# revision 11
# speedup vs baseline: 1.5648x; 1.5648x over previous
"""Bass/Trainium2 kernel for nn_GCL_49959059587771 (GCL JSD loss).

Math: reference computes, for pair (z, g, batch):
    zn = z / max(||z||, eps);  gn = g / max(||g||, eps)
    self_sim  = (zn @ gn_self.T)  * onehot(batch)   # [N, G]
    cross_sim = (zn @ gn_cross.T) * onehot(batch)
    d = ep_jsd(self_sim).sum(1) - ep_jsd(cross_sim).sum(1)
    L = sqrt(sum(d^2))
where ep_jsd(x) = log2 - softplus(-x), and ep_jsd(0) = 0 exactly. The
one-hot mask therefore collapses each row of the [N, G] matrices to a
single entry: the masked row-sum of ep_jsd equals ep_jsd of the one
gathered dot product (all other entries are ep_jsd(0) = 0, and even their
shared constant would cancel in the self-cross difference).

So per node i:
    s_i = <z_i, gn_self[b_i]>  / ||z_i||
    c_i = <z_i, gn_cross[b_i]> / ||z_i||
    d_i = softplus(-c_i) - softplus(-s_i)
and the answer is sqrt(sum d1^2) + sqrt(sum d2^2).

Strategy (8 NeuronCores, SPMD, node-parallel):
  - shard nodes N across 8 cores (6250 each, padded to 6272 = 49*128)
  - replicate g (concatenated [g1 | g2] rows -> one 512-col "cat row")
  - on device: normalize g rows once, store to DRAM scratch, then
    dma_gather cat rows per node; per 128-node tile use the fused DVE
    tensor_tensor_reduce (multiply + free-axis reduce) for the two dot
    products and ACT Square+accum for ||z||^2; tiny [128, 49] epilogue
    does the normalize / softplus / d^2 accumulation via Exp/Ln.
  - per-core output: [128, 2] partial sums of d1^2 / d2^2; host finishes
    the all-reduce (sum over cores+partitions), sqrt, add.
"""

import numpy as np
from contextlib import ExitStack

import concourse.bass as bass
import concourse.bacc as bacc
import concourse.tile as tile
import concourse.mybir as mybir
from concourse.bass_utils import run_bass_kernel_spmd

N, G, D = 50000, 512, 256
NCORES = 8
RPC = N // NCORES            # 6250 rows per core
NT = 49                      # 128-row tiles per core
RPAD = NT * 128              # 6272
GRP = 7                      # tiles per gather/DMA group
NGRP = NT // GRP             # 7

AF = mybir.ActivationFunctionType
ALU = mybir.AluOpType
F32 = mybir.dt.float32
I16 = mybir.dt.int16
BF16 = mybir.dt.bfloat16

# compute dtype for z tiles and normalized-g gather payloads
Z_DT = BF16
G_DT = BF16
_NP_Z = {F32: np.float32, BF16: "bfloat16"}


def build(z_dt=Z_DT, g_dt=G_DT, debug=False):
    nc = bacc.Bacc("TRN2", target_bir_lowering=False, debug=debug)

    z1t = nc.dram_tensor("z1t", [128, NT, D], z_dt, kind="ExternalInput")
    z2t = nc.dram_tensor("z2t", [128, NT, D], z_dt, kind="ExternalInput")
    # one-hot routing matrices: oh[v_local, t, p] = 1 iff node t*128+p has
    # (windowed) batch value v_local.  Gather becomes OH.T @ Gwin on TensorE.
    oh1 = nc.dram_tensor("oh1", [128, NT, 128], g_dt, kind="ExternalInput")
    oh2 = nc.dram_tensor("oh2", [128, NT, 128], g_dt, kind="ExternalInput")
    # per-core 128-value window of [g1 | g2] cat rows (raw; device normalizes)
    gwin = nc.dram_tensor("gwin", [128, 2 * D], F32, kind="ExternalInput")
    acc = nc.dram_tensor("acc", [128, 2], F32, kind="ExternalOutput")

    with tile.TileContext(nc) as tc, ExitStack() as ctx:
        gpool = ctx.enter_context(tc.tile_pool(name="gnorm", bufs=2))
        singles = ctx.enter_context(tc.tile_pool(name="singles", bufs=1))
        zpool = ctx.enter_context(tc.tile_pool(name="z", bufs=3))
        ggpool = ctx.enter_context(tc.tile_pool(name="gg", bufs=4,
                                                space="PSUM"))
        junk = ctx.enter_context(tc.tile_pool(name="junk", bufs=6))
        small = ctx.enter_context(tc.tile_pool(name="small", bufs=4))

        # ---- phase 1: row-normalize the g window; keep resident in SBUF ----
        g_in = gpool.tile([128, 2 * D], F32, tag="g_in")
        nc.sync.dma_start(g_in[:], gwin[:])
        gn2 = small.tile([128, 2], F32, tag="gn2")
        for h in range(2):
            sq = junk.tile([128, D], F32, tag="junk")
            nc.vector.scalar_tensor_tensor(
                out=sq[:], in0=g_in[:, h * D:(h + 1) * D], scalar=1.0,
                in1=g_in[:, h * D:(h + 1) * D],
                op0=ALU.mult, op1=ALU.mult, accum_out=gn2[:, h:h + 1])
        # 1/sqrt(n2) = exp(-0.5 * ln(n2)); g norms are O(16), no eps issue
        ginv = small.tile([128, 2], F32, tag="ginv")
        nc.scalar.activation(out=ginv[:], in_=gn2[:], func=AF.Ln)
        nc.scalar.activation(out=ginv[:], in_=ginv[:], func=AF.Exp, scale=-0.5)
        gnorm = singles.tile([128, 2 * D], g_dt)
        for h in range(2):
            nc.vector.tensor_scalar_mul(
                gnorm[:, h * D:(h + 1) * D], g_in[:, h * D:(h + 1) * D],
                ginv[:, h:h + 1])

        # ---- per-tile accumulators ----
        r1s = singles.tile([128, NT], F32)
        r1c = singles.tile([128, NT], F32)
        r2s = singles.tile([128, NT], F32)
        r2c = singles.tile([128, NT], F32)
        nn1 = singles.tile([128, NT], F32)
        nn2 = singles.tile([128, NT], F32)

        # ---- main loop ----
        for grp in range(NGRP):
            z1c = zpool.tile([128, GRP, D], z_dt, tag="z1c")
            nc.sync.dma_start(z1c[:], z1t[:, grp * GRP:(grp + 1) * GRP, :])
            z2c = zpool.tile([128, GRP, D], z_dt, tag="z2c")
            nc.sync.dma_start(z2c[:], z2t[:, grp * GRP:(grp + 1) * GRP, :])
            oh1c = zpool.tile([128, GRP, 128], g_dt, tag="oh1c")
            nc.sync.dma_start(oh1c[:], oh1[:, grp * GRP:(grp + 1) * GRP, :])
            oh2c = zpool.tile([128, GRP, 128], g_dt, tag="oh2c")
            nc.sync.dma_start(oh2c[:], oh2[:, grp * GRP:(grp + 1) * GRP, :])
            for tt in range(GRP):
                t = grp * GRP + tt
                for (zc, ohc, rs, rc, nn) in ((z1c, oh1c, r1s, r1c, nn1),
                                              (z2c, oh2c, r2s, r2c, nn2)):
                    # gather normalized cat rows: gg = OH.T @ gnorm (PSUM)
                    gg = ggpool.tile([128, 2 * D], F32, tag="gg")
                    nc.tensor.matmul(gg[:], ohc[:, tt, :], gnorm[:],
                                     start=True, stop=True)
                    # self dot: pair 1 uses g1n (cols 0:D), pair 2 uses g2n
                    sh, ch = (0, D) if zc is z1c else (D, 0)
                    js = junk.tile([128, D], z_dt, tag="junk")
                    nc.vector.scalar_tensor_tensor(
                        out=js[:], in0=zc[:, tt, :], scalar=1.0,
                        in1=gg[:, sh:sh + D],
                        op0=ALU.mult, op1=ALU.mult, accum_out=rs[:, t:t + 1])
                    jc = junk.tile([128, D], z_dt, tag="junk")
                    nc.vector.scalar_tensor_tensor(
                        out=jc[:], in0=zc[:, tt, :], scalar=1.0,
                        in1=gg[:, ch:ch + D],
                        op0=ALU.mult, op1=ALU.mult, accum_out=rc[:, t:t + 1])
                    jn = junk.tile([128, D], F32, tag="junk")
                    nc.scalar.activation(out=jn[:], in_=zc[:, tt, :],
                                         func=AF.Square,
                                         accum_out=nn[:, t:t + 1])

        # ---- epilogue on [128, NT] ----
        # inv_norm = exp(-0.5*ln(n2 + eps));  eps keeps padded zero rows finite
        eps_b = singles.tile([128, 1], F32)
        nc.vector.memset(eps_b[:], 1e-12)
        inv1 = singles.tile([128, NT], F32)
        nc.scalar.activation(out=inv1[:], in_=nn1[:], func=AF.Ln, bias=eps_b[:])
        nc.scalar.activation(out=inv1[:], in_=inv1[:], func=AF.Exp, scale=-0.5)
        inv2 = singles.tile([128, NT], F32)
        nc.scalar.activation(out=inv2[:], in_=nn2[:], func=AF.Ln, bias=eps_b[:])
        nc.scalar.activation(out=inv2[:], in_=inv2[:], func=AF.Exp, scale=-0.5)

        acc_sb = singles.tile([128, 2], F32)
        for j, (rs, rc, inv) in enumerate(((r1s, r1c, inv1), (r2s, r2c, inv2))):
            s = small.tile([128, NT], F32, tag="s")
            nc.vector.tensor_mul(s[:], rs[:], inv[:])
            c = small.tile([128, NT], F32, tag="c")
            nc.vector.tensor_mul(c[:], rc[:], inv[:])
            # softplus(-x) = ln(1 + exp(-x))
            sp_s = small.tile([128, NT], F32, tag="sp_s")
            nc.scalar.activation(out=sp_s[:], in_=s[:], func=AF.Exp, scale=-1.0)
            nc.scalar.activation(out=sp_s[:], in_=sp_s[:], func=AF.Ln, bias=1.0)
            sp_c = small.tile([128, NT], F32, tag="sp_c")
            nc.scalar.activation(out=sp_c[:], in_=c[:], func=AF.Exp, scale=-1.0)
            nc.scalar.activation(out=sp_c[:], in_=sp_c[:], func=AF.Ln, bias=1.0)
            d = small.tile([128, NT], F32, tag="d")
            nc.vector.tensor_sub(d[:], sp_c[:], sp_s[:])
            jd = junk.tile([128, NT], F32, tag="jd")
            nc.scalar.activation(out=jd[:], in_=d[:], func=AF.Square,
                                 accum_out=acc_sb[:, j:j + 1])
        nc.sync.dma_start(acc[:], acc_sb[:])

    nc.compile()
    return nc


_prog = None


def _get_prog():
    global _prog
    if _prog is None:
        _prog = build()
    return _prog


def _prep_inputs(z1, z2, g1, g2, batch_1, batch_2):
    import ml_dtypes  # noqa: F401  (registers bfloat16 with numpy)
    z1 = np.asarray(z1, dtype=np.float32)
    z2 = np.asarray(z2, dtype=np.float32)
    b1 = np.asarray(batch_1).astype(np.int64).ravel()
    b2 = np.asarray(batch_2).astype(np.int64).ravel()
    gcat = np.concatenate([np.asarray(g1, np.float32),
                           np.asarray(g2, np.float32)], axis=1)  # [G, 2D]
    z_np = np.dtype("float32") if Z_DT == F32 else np.dtype("bfloat16")
    g_np = np.dtype("float32") if G_DT == F32 else np.dtype("bfloat16")

    in_maps = []
    for k in range(NCORES):
        sl = slice(k * RPC, (k + 1) * RPC)

        def prep_z(z):
            zs = np.zeros((RPAD, D), np.float32)
            zs[:RPC] = z[sl]
            zt = zs.reshape(NT, 128, D).transpose(1, 0, 2)
            return np.ascontiguousarray(zt.astype(z_np))

        # shared 128-value window for this core (both batches index g rows)
        v0 = int(min(b1[sl].min(), b2[sl].min()))
        vhi = int(max(b1[sl].max(), b2[sl].max()))
        assert vhi - v0 < 128, f"core {k}: value span {vhi - v0 + 1} > 128"
        gw = np.zeros((128, 2 * D), np.float32)
        nrows = min(128, G - v0)
        gw[:nrows] = gcat[v0:v0 + nrows]
        gw[nrows:] = 1.0  # never-selected pad rows; keep norms finite

        def prep_oh(b):
            bl = (b[sl] - v0).astype(np.int64)          # [RPC] in [0,128)
            oh = np.zeros((128, RPAD), np.float32)      # [v_local, node]
            oh[bl, np.arange(RPC)] = 1.0
            oh = oh.reshape(128, NT, 128)
            return np.ascontiguousarray(oh.astype(g_np))

        in_maps.append({"z1t": prep_z(z1), "z2t": prep_z(z2),
                        "oh1": prep_oh(b1), "oh2": prep_oh(b2),
                        "gwin": np.ascontiguousarray(gw)})
    return in_maps


def _finish(results):
    tot = np.zeros(2, np.float64)
    for r in results:
        tot += r["acc"].astype(np.float64).sum(axis=0)
    return np.float32(np.sqrt(tot[0]) + np.sqrt(tot[1]))


def kernel(z1, z2, g1, g2, batch_1, batch_2, trace=False):
    nc = _get_prog()
    in_maps = _prep_inputs(z1, z2, g1, g2, batch_1, batch_2)
    res = run_bass_kernel_spmd(nc, in_maps, core_ids=list(range(NCORES)),
                               trace=trace)
    out = _finish(res.results)
    if trace:
        kernel.last_results = res
    return out
